# revision 26
# baseline (speedup 1.0000x reference)
"""ASTER decoder (attention + GRU, teacher-forced) Trainium2 Bass kernel.

Data-parallel over B=256 -> 8 cores x 32 rows. Per core:
  - one-time: P = enc @ w_enc + b_att in [k-part, (b,t)] layout
  - 28 sequential steps: q -> S=tanh(P+q) -> scores (v-dot) -> softmax
    -> ctx (block-diag alpha matmul) -> GRU gates -> h_new
  - end: logits = h_all @ W_out.T + b_out batched over all 28 steps
"""
import sys

sys.path.insert(0, "/opt/trn_rl_repo")

import numpy as np
import ml_dtypes

from concourse import bass, mybir
from concourse.tile import TileContext
from concourse.bass_utils import run_bass_kernel_spmd

T, B, H, E, C, L = 64, 256, 512, 256, 6625, 28
NCORE = 8
BS = B // NCORE          # 32 rows per core
NBT = BS * T             # 2048 (b,t) pairs, b-major
NT = 13                  # 13 n-tiles of 512 -> 6656 padded classes
CP = NT * 512

f32 = mybir.dt.float32
f32r = mybir.dt.float32r
bf16 = mybir.dt.bfloat16
Act = mybir.ActivationFunctionType
Alu = mybir.AluOpType

_CACHE = {}


def _split_waits_json(nc):
    """Walrus/ISA allows one sync-wait per instruction; Tile may emit
    several. Hoist extras onto NoOps inserted just before the instruction
    on the same engine."""
    import json as _json
    js = _json.loads(nc.to_json_bytes())
    nid = 900000
    for fn in js["functions"]:
        for bb in fn["blocks"]:
            out = []
            for inst in bb["instructions"]:
                si = inst.get("sync_info")
                waits = (si or {}).get("on_wait") or []
                if len(waits) > 1:
                    for w in waits[:-1]:
                        nid += 1
                        out.append({
                            "debug": inst.get("debug", 0),
                            "engine": inst["engine"], "ins": [],
                            "name": f"I-{nid}", "opcode": "NoOp",
                            "outs": [],
                            "sync_info": {"on_wait": [w], "on_update": []},
                        })
                    si["on_wait"] = [waits[-1]]
                out.append(inst)
            bb["instructions"] = out
    return _json.dumps(js).encode()


def _build():
    nc = bass.Bass()

    def dp(name, shape, dt, out=False):
        return nc.declare_dram_parameter(name, list(shape), dt, isOutput=out)

    d_encT = dp("encT", (128, 4, NBT), bf16)       # [p, hc, b*64+t]
    d_encS = dp("encS", (128, 16, H), bf16)       # [(bp,t), b2, h]
    d_embT = dp("embT", (128, 2, L, BS), bf16)    # [p, ec, l, b]
    d_wenc = dp("wenc", (128, 4, H), bf16)         # lhsT w_enc[h, k]
    d_whid = dp("whid", (128, 4, H), bf16)        # lhsT w_hid[h, k]
    d_batt = dp("batt", (128, 4), f32)
    d_v4 = dp("v4", (128, 4, 32), bf16)
    d_WihT = dp("WihT", (128, 6, 1536), bf16)     # [f-part, fc, j]
    d_WhhT = dp("WhhT", (128, 4, 1536), bf16)
    d_brz = dp("brz", (128, 8), f32)
    d_bin = dp("bin_", (128, 4), f32)
    d_bhn = dp("bhn", (128, 4), f32)
    d_h0T = dp("h0T", (128, 4, BS), f32)
    d_WoT = dp("WoT", (128, 4, CP), bf16)         # [p, kc, n] = W_out[n, kc*128+p]
    d_id64 = dp("id64", (64, 64), f32)
    d_out = dp("out", (L * BS, CP), f32, out=True)
    d_scb = [nc.dram_tensor(f"scb{i}", [128, 512], f32) for i in range(2)]

    with TileContext(nc) as tc:
        with tc.tile_pool(name="const", bufs=1) as cpool, \
             tc.tile_pool(name="state", bufs=1) as spool, \
             tc.tile_pool(name="work", bufs=2) as wpool, \
             tc.tile_pool(name="wstream", bufs=2) as wspool, \
             tc.tile_pool(name="ostream", bufs=3) as opool, \
             tc.tile_pool(name="ps", bufs=1, space="PSUM") as ps:

            # ---------- load constants ----------
            def ld(dram, dt=None, pool=cpool):
                t_ = pool.tile(dram.shape, dt or dram.dtype, name="c_" + dram.name)
                nc.sync.dma_start(out=t_, in_=dram[:])
                return t_

            encS = ld(d_encS)
            embT = ld(d_embT)
            wenc = ld(d_wenc)
            whid = ld(d_whid)
            batt = ld(d_batt)
            v4 = ld(d_v4)
            WihT = ld(d_WihT)
            WhhT = ld(d_WhhT)
            brz = ld(d_brz)
            bin_ = ld(d_bin)
            bhn = ld(d_bhn)

            id64 = ld(d_id64)

            # ---------- persistent state ----------
            P_sb = [spool.tile([128, NBT], bf16, name=f"P_sb{i}")
                    for i in range(4)]
            S_sb = [spool.tile([128, NBT], bf16, name=f"S_sb{i}")
                    for i in range(4)]
            hT = spool.tile([128, 4, BS], f32, name="hT")
            hTb = spool.tile([128, 4, BS], bf16, name="hTb")
            hall = spool.tile([128, 4, L * BS], bf16, name="hall")
            Ablk = spool.tile([128, 512], bf16, name="Ablk")
            nc.vector.memset(Ablk, 0.0)
            nc.sync.dma_start(out=hT, in_=d_h0T[:])
            nc.vector.tensor_copy(hTb, hT)

            # ---------- one-time: P.T = w_enc.T-blocks @ encT (+ b_att) ----------
            for ns in range(4):
                ect = wspool.tile([128, 4, 512], bf16, name="ect")
                nc.sync.dma_start(out=ect, in_=d_encT[:, :, ns * 512:(ns + 1) * 512])
                for kc in range(4):
                    pt = ps.tile([128, 512], f32, name="ppt", tag="big", bufs=2)
                    for hc in range(4):
                        nc.tensor.matmul(
                            pt,
                            wenc[:, hc, kc * 128:(kc + 1) * 128],
                            ect[:, hc, :],
                            start=(hc == 0), stop=(hc == 3),
                        )
                    nc.scalar.activation(
                        P_sb[kc][:, ns * 512:(ns + 1) * 512], pt,
                        Act.Identity, bias=batt[:, kc:kc + 1], scale=1.0)

            # ---------- recurrence ----------
            opend = []
            for l in range(L):
                # q.T[k, b] = w_hid-blocks @ hTb : PSUM [128,(4,32)]
                qp = ps.tile([128, 4, BS], f32, name="qp", tag="qc", bufs=2)
                for kc in range(4):
                    for hc in range(4):
                        nc.tensor.matmul(
                            qp[:, kc, :],
                            whid[:, hc, kc * 128:(kc + 1) * 128],
                            hTb[:, hc, :],
                            start=(hc == 0), stop=(hc == 3))
                q_sb = wpool.tile([128, 4, BS], bf16, name="q_sb")
                nc.vector.tensor_copy(q_sb, qp)

                # S = tanh(P + q) ; bcast q over t via stride-0 AP
                for kc in range(4):
                    qb = q_sb[:, kc, None, :].broadcast_to([128, T, BS])
                    nc.vector.tensor_tensor(
                        S_sb[kc].rearrange("p (t b) -> p t b", b=BS),
                        P_sb[kc].rearrange("p (t b) -> p t b", b=BS),
                        qb, Alu.add)
                    nc.scalar.activation(
                        S_sb[kc], S_sb[kc], Act.Tanh,
                        bias=0.0, scale=1.0)

                # scores: v-dot, 4 col-group rows of PSUM
                scp = ps.tile([128, 512], f32, name="scp", tag="scp", bufs=1)
                for ns in range(4):
                    for kc in range(4):
                        nc.tensor.matmul(
                            scp[32 * ns:32 * ns + 32, :],
                            v4[:, kc, :],
                            S_sb[kc][:, ns * 512:(ns + 1) * 512],
                            start=(kc == 0), stop=(kc == 3),
                            tile_position=(0, 32 * ns))
                sc128 = wpool.tile([128, 512], f32, name="sc128")
                nc.vector.tensor_copy(sc128, scp)
                scb = d_scb[l % 2]
                nc.sync.dma_start(out=scb[:], in_=sc128)
                dview = scb[:].rearrange("(n r) (tt b) -> n r tt b",
                                         r=32, b=BS)[:, 0]
                sctb = wpool.tile([T, BS], f32, name="sctb")
                nc.sync.dma_start(out=sctb, in_=dview)
                trp = ps.tile([32, T], f32, name="trp", tag="qc", bufs=2)
                nc.tensor.transpose(trp, sctb, id64)
                scbt = wpool.tile([32, T], f32, name="scbt")
                nc.vector.tensor_copy(scbt, trp)

                # softmax over t (free dim), per-partition b
                nmax = wpool.tile([32, 1], f32, name="nmax")
                nc.vector.tensor_reduce(nmax, scbt, mybir.AxisListType.X,
                                        Alu.max, negate=True)
                wbt = wpool.tile([32, T], f32, name="wbt")
                zsum = wpool.tile([32, 1], f32, name="zsum")
                nc.scalar.activation(wbt, scbt, Act.Exp, bias=nmax, scale=1.0,
                                     accum_out=zsum)
                rz = wpool.tile([32, 1], f32, name="rz")
                nc.vector.reciprocal(rz, zsum)
                albt = wpool.tile([32, T], bf16, name="albt")
                nc.vector.tensor_scalar(albt, wbt, rz, None, Alu.mult)

                # alphaT [64, 32] via two 32x32 DVE transposes
                alT = wpool.tile([64, 32], bf16, name="alT")
                nc.vector.transpose(alT[0:32, :], albt[:, 0:32])
                nc.vector.transpose(alT[32:64, :], albt[:, 32:64])

                # scatter alphaT into block-diag Ablk (zeros persist)
                nc.vector.tensor_copy(Ablk[0:64, 0:512:34], alT[:, 0::2])
                nc.vector.tensor_copy(Ablk[64:128, 1:512:34], alT[:, 1::2])

                # ctx.T [h-chunk, b] = encS-chunks.T @ Ablk-cols
                cxp = ps.tile([128, 4, BS], f32, name="cxp", tag="qc", bufs=2)
                for mc in range(4):
                    for c in range(16):
                        nc.tensor.matmul(
                            cxp[:, mc, :],
                            encS[:, c, mc * 128:(mc + 1) * 128],
                            Ablk[:, c * 32:(c + 1) * 32],
                            start=(c == 0), stop=(c == 15))

                # xT = [ctx.T (bf16) ; embT_l]
                xT = wpool.tile([128, 6, BS], bf16, name="xT")
                nc.vector.tensor_copy(xT[:, 0:4, :], cxp)
                nc.vector.tensor_copy(xT[:, 4:6, :], embT[:, :, l, :])

                # GRU gates: grz = (Wih@x + Whh@h) for r,z ; gin/ghn separate
                grzp = ps.tile([128, 8, BS], f32, name="grzp", tag="grz", bufs=1)
                ginp = ps.tile([128, 4, BS], f32, name="ginp", tag="gin", bufs=1)
                ghnp = ps.tile([128, 4, BS], f32, name="ghnp", tag="ghn", bufs=1)
                for jc in range(8):
                    for fc in range(6):
                        nc.tensor.matmul(
                            grzp[:, jc, :],
                            WihT[:, fc, jc * 128:(jc + 1) * 128],
                            xT[:, fc, :], start=(fc == 0), stop=False)
                    for hc in range(4):
                        nc.tensor.matmul(
                            grzp[:, jc, :],
                            WhhT[:, hc, jc * 128:(jc + 1) * 128],
                            hTb[:, hc, :], start=False, stop=(hc == 3))
                for jc in range(4):
                    for fc in range(6):
                        nc.tensor.matmul(
                            ginp[:, jc, :],
                            WihT[:, fc, (8 + jc) * 128:(9 + jc) * 128],
                            xT[:, fc, :], start=(fc == 0), stop=(fc == 5))
                    for hc in range(4):
                        nc.tensor.matmul(
                            ghnp[:, jc, :],
                            WhhT[:, hc, (8 + jc) * 128:(9 + jc) * 128],
                            hTb[:, hc, :], start=(hc == 0), stop=(hc == 3))

                # gate nonlinearities (bias adds on DVE, single wide ACTs)
                grzs = wpool.tile([128, 8, BS], f32, name="grzs")
                nc.vector.tensor_tensor(
                    grzs, grzp, brz[:, :, None].broadcast_to([128, 8, BS]),
                    Alu.add)
                rzs = wpool.tile([128, 8, BS], bf16, name="rzs")
                nc.scalar.activation(rzs, grzs, Act.Sigmoid, bias=0.0,
                                     scale=1.0)
                gins = wpool.tile([128, 4, BS], f32, name="gins")
                ghns = wpool.tile([128, 4, BS], f32, name="ghns")
                nc.vector.tensor_tensor(
                    gins, ginp, bin_[:, :, None].broadcast_to([128, 4, BS]),
                    Alu.add)
                nc.vector.tensor_tensor(
                    ghns, ghnp, bhn[:, :, None].broadcast_to([128, 4, BS]),
                    Alu.add)
                npre = wpool.tile([128, 4, BS], f32, name="npre")
                nc.vector.tensor_tensor(npre, rzs[:, 0:4, :], ghns, Alu.mult)
                nc.vector.tensor_tensor(npre, npre, gins, Alu.add)
                nsb = wpool.tile([128, 4, BS], f32, name="nsb")
                nc.scalar.activation(nsb, npre, Act.Tanh, bias=0.0, scale=1.0)

                # h = n + z*(h - n)
                hmn = wpool.tile([128, 4, BS], f32, name="hmn")
                nc.vector.tensor_tensor(hmn, hT, nsb, Alu.subtract)
                nc.vector.tensor_tensor(hmn, rzs[:, 4:8, :], hmn, Alu.mult)
                nc.vector.tensor_tensor(hT, nsb, hmn, Alu.add)
                nc.vector.tensor_copy(hTb, hT)
                nc.vector.tensor_copy(hall[:, :, l * BS:(l + 1) * BS], hT)

                # interleaved out-projection, spread ~4 tiles per step
                if l % 4 == 3:
                    opend.extend((l // 4, nt) for nt in range(NT))
                burst = 4 if l < L - 1 else len(opend)
                for mc, nt in [opend.pop(0) for _ in range(min(burst, len(opend)))]:
                    wt = wspool.tile([128, 4, 512], bf16, name="wt")
                    nc.sync.dma_start(
                        out=wt, in_=d_WoT[:, :, nt * 512:(nt + 1) * 512])
                    op = ps.tile([128, 512], f32, name="op",
                                 tag="big", bufs=2)
                    for kc in range(4):
                        nc.tensor.matmul(
                            op,
                            hall[:, kc, mc * 128:(mc + 1) * 128],
                            wt[:, kc, :],
                            start=(kc == 0), stop=(kc == 3))
                    ot = opool.tile([128, 512], f32, name="ot")
                    nc.vector.tensor_copy(ot, op)
                    nc.sync.dma_start(
                        out=d_out[mc * 128:(mc + 1) * 128,
                                  nt * 512:(nt + 1) * 512],
                        in_=ot)

    return nc


def _prep_inputs(inputs):
    """Host-side: shard + lay out per-core DRAM param arrays."""
    enc = np.asarray(inputs["encoded"], np.float32)
    brg = np.asarray(inputs["encoder_bridge"], np.float32)
    tgt = np.asarray(inputs["targets"])
    emb = np.asarray(inputs["embedding"], np.float32)
    w_enc = np.asarray(inputs["w_enc"], np.float32)
    w_hid = np.asarray(inputs["w_hid"], np.float32)
    b_att = np.asarray(inputs["b_att"], np.float32)
    v_att = np.asarray(inputs["v_att"], np.float32)
    W_ih = np.asarray(inputs["W_ih"], np.float32)
    W_hh = np.asarray(inputs["W_hh"], np.float32)
    b_ih = np.asarray(inputs["b_ih"], np.float32)
    b_hh = np.asarray(inputs["b_hh"], np.float32)
    W_out = np.asarray(inputs["W_out"], np.float32)
    b_out = np.asarray(inputs["b_out"], np.float32)
    W_br = np.asarray(inputs["W_bridge"], np.float32)
    b_br = np.asarray(inputs["b_bridge"], np.float32)

    bf = ml_dtypes.bfloat16
    # shared (replicated) params
    wenc = np.ascontiguousarray(w_enc.reshape(4, 128, H).transpose(1, 0, 2)).astype(bf)
    whid = np.ascontiguousarray(w_hid.reshape(4, 128, H).transpose(1, 0, 2)).astype(bf)
    batt = np.ascontiguousarray(b_att.reshape(4, 128).T)
    v4 = np.ascontiguousarray(
        np.broadcast_to(v_att.reshape(4, 128).T[:, :, None],
                        (128, 4, 32))).astype(bf)
    WihT = np.ascontiguousarray(W_ih.T.reshape(6, 128, 1536).transpose(1, 0, 2)).astype(bf)
    WhhT = np.ascontiguousarray(W_hh.T.reshape(4, 128, 1536).transpose(1, 0, 2)).astype(bf)
    brz_f = (b_ih + b_hh)[:1024]
    brz = np.ascontiguousarray(brz_f.reshape(8, 128).T)
    bin_ = np.ascontiguousarray(b_ih[1024:].reshape(4, 128).T)
    bhn = np.ascontiguousarray(b_hh[1024:].reshape(4, 128).T)
    Wp = np.zeros((CP, H), np.float32)
    Wp[:C] = W_out
    WoT = np.ascontiguousarray(Wp.T.reshape(4, 128, CP).transpose(1, 0, 2)).astype(bf)
    id64 = np.eye(64, dtype=np.float32)

    # teacher-forced inputs + bridge (host)
    xs = np.concatenate([np.zeros((1, B), tgt.dtype), tgt[:, 1:L].T], 0)  # (L,B)
    emb_seq = emb[xs]                                    # (L,B,E)
    h0 = np.tanh(brg @ W_br.T + b_br)                    # (B,H)

    in_maps = []
    for ci in range(NCORE):
        sl = slice(ci * BS, (ci + 1) * BS)
        e = enc[:, sl, :]                                # (T,32,H)
        encTc = np.ascontiguousarray(
            e.transpose(2, 0, 1).reshape(H, NBT).reshape(4, 128, NBT)
            .transpose(1, 0, 2)).astype(bf)
        encSc = np.ascontiguousarray(
            e.transpose(1, 0, 2).reshape(16, 2, T, H).transpose(1, 2, 0, 3)
            .reshape(128, 16, H)).astype(bf)
        embTc = np.ascontiguousarray(
            emb_seq[:, sl, :].transpose(2, 0, 1).reshape(2, 128, L, BS)
            .transpose(1, 0, 2, 3)).astype(bf)
        h0Tc = np.ascontiguousarray(
            h0[sl].T.reshape(4, 128, BS).transpose(1, 0, 2))
        in_maps.append({
            "encT": encTc, "encS": encSc, "embT": embTc,
            "wenc": wenc, "whid": whid, "batt": batt, "v4": v4,
            "WihT": WihT, "WhhT": WhhT, "brz": brz, "bin_": bin_,
            "bhn": bhn, "h0T": h0Tc,
            "WoT": WoT, "id64": id64,
        })
    return in_maps


def kernel(**inputs):
    if "nc" not in _CACHE:
        nc = _build()
        fixed = _split_waits_json(nc)
        nc.to_json_bytes = lambda: fixed
        _CACHE["nc"] = nc
    nc = _CACHE["nc"]
    in_maps = _prep_inputs(inputs)
    res = run_bass_kernel_spmd(nc, in_maps, list(range(NCORE)))
    outs = []
    for ci in range(NCORE):
        o = np.asarray(res.results[ci]["out"])       # (L*BS, CP)
        outs.append(o.reshape(L, BS, CP)[:, :, :C].transpose(1, 0, 2))
    full = np.concatenate(outs, 0).astype(np.float32)  # (B, L, C)
    return full + np.asarray(inputs["b_out"], np.float32)[None, None, :]


# revision 27
# speedup vs baseline: 1.0135x; 1.0135x over previous
"""ASTER decoder (attention + GRU, teacher-forced) Trainium2 Bass kernel.

Data-parallel over B=256 -> 8 cores x 32 rows. Per core:
  - one-time: P = enc @ w_enc + b_att in [k-part, (b,t)] layout
  - 28 sequential steps: q -> S=tanh(P+q) -> scores (v-dot) -> softmax
    -> ctx (block-diag alpha matmul) -> GRU gates -> h_new
  - end: logits = h_all @ W_out.T + b_out batched over all 28 steps
"""
import sys

sys.path.insert(0, "/opt/trn_rl_repo")

import numpy as np
import ml_dtypes

from concourse import bass, mybir
from concourse.tile import TileContext
from concourse.bass_utils import run_bass_kernel_spmd

T, B, H, E, C, L = 64, 256, 512, 256, 6625, 28
NCORE = 8
BS = B // NCORE          # 32 rows per core
NBT = BS * T             # 2048 (b,t) pairs, b-major
NT = 13                  # 13 n-tiles of 512 -> 6656 padded classes
CP = NT * 512

f32 = mybir.dt.float32
f32r = mybir.dt.float32r
bf16 = mybir.dt.bfloat16
Act = mybir.ActivationFunctionType
Alu = mybir.AluOpType

_CACHE = {}


def _split_waits_json(nc):
    """Walrus/ISA allows one sync-wait per instruction; Tile may emit
    several. Hoist extras onto NoOps inserted just before the instruction
    on the same engine."""
    import json as _json
    js = _json.loads(nc.to_json_bytes())
    nid = 900000
    for fn in js["functions"]:
        for bb in fn["blocks"]:
            out = []
            for inst in bb["instructions"]:
                si = inst.get("sync_info")
                waits = (si or {}).get("on_wait") or []
                if len(waits) > 1:
                    for w in waits[:-1]:
                        nid += 1
                        out.append({
                            "debug": inst.get("debug", 0),
                            "engine": inst["engine"], "ins": [],
                            "name": f"I-{nid}", "opcode": "NoOp",
                            "outs": [],
                            "sync_info": {"on_wait": [w], "on_update": []},
                        })
                    si["on_wait"] = [waits[-1]]
                out.append(inst)
            bb["instructions"] = out
    return _json.dumps(js).encode()


def _build():
    nc = bass.Bass()

    def dp(name, shape, dt, out=False):
        return nc.declare_dram_parameter(name, list(shape), dt, isOutput=out)

    d_encT = dp("encT", (128, 4, NBT), bf16)       # [p, hc, b*64+t]
    d_encS = dp("encS", (128, 16, H), bf16)       # [(bp,t), b2, h]
    d_embT = dp("embT", (128, 2, L, BS), bf16)    # [p, ec, l, b]
    d_wenc = dp("wenc", (128, 4, H), bf16)         # lhsT w_enc[h, k]
    d_whid = dp("whid", (128, 4, H), bf16)        # lhsT w_hid[h, k]
    d_batt = dp("batt", (128, 4), f32)
    d_v4 = dp("v4", (128, 4, 32), bf16)
    d_WihT = dp("WihT", (128, 6, 1536), bf16)     # [f-part, fc, j]
    d_WhhT = dp("WhhT", (128, 4, 1536), bf16)
    d_brz = dp("brz", (128, 8), f32)
    d_bin = dp("bin_", (128, 4), f32)
    d_bhn = dp("bhn", (128, 4), f32)
    d_h0T = dp("h0T", (128, 4, BS), f32)
    d_WoT = dp("WoT", (128, 4, CP), bf16)         # [p, kc, n] = W_out[n, kc*128+p]
    d_id64 = dp("id64", (64, 64), f32)
    d_out = dp("out", (L * BS, CP), f32, out=True)
    d_scb = [nc.dram_tensor(f"scb{i}", [128, 512], f32) for i in range(2)]

    with TileContext(nc) as tc:
        with tc.tile_pool(name="const", bufs=1) as cpool, \
             tc.tile_pool(name="state", bufs=1) as spool, \
             tc.tile_pool(name="work", bufs=2) as wpool, \
             tc.tile_pool(name="wstream", bufs=2) as wspool, \
             tc.tile_pool(name="ostream", bufs=3) as opool, \
             tc.tile_pool(name="ps", bufs=1, space="PSUM") as ps:

            # ---------- load constants ----------
            def ld(dram, dt=None, pool=cpool):
                t_ = pool.tile(dram.shape, dt or dram.dtype, name="c_" + dram.name)
                nc.sync.dma_start(out=t_, in_=dram[:])
                return t_

            encS = ld(d_encS)
            embT = ld(d_embT)
            wenc = ld(d_wenc)
            whid = ld(d_whid)
            batt = ld(d_batt)
            v4 = ld(d_v4)
            WihT = ld(d_WihT)
            WhhT = ld(d_WhhT)
            brz = ld(d_brz)
            bin_ = ld(d_bin)
            bhn = ld(d_bhn)

            id64 = ld(d_id64)

            # ---------- persistent state ----------
            P_sb = [spool.tile([128, NBT], bf16, name=f"P_sb{i}")
                    for i in range(4)]
            S_sb = [spool.tile([128, NBT], bf16, name=f"S_sb{i}")
                    for i in range(4)]
            hT = spool.tile([128, 4, BS], f32, name="hT")
            hTb = spool.tile([128, 4, BS], bf16, name="hTb")
            hall = spool.tile([128, 4, L * BS], bf16, name="hall")
            Ablk = spool.tile([128, 512], bf16, name="Ablk")
            nc.vector.memset(Ablk, 0.0)
            nc.sync.dma_start(out=hT, in_=d_h0T[:])
            nc.vector.tensor_copy(hTb, hT)

            # ---------- one-time: P.T = w_enc.T-blocks @ encT (+ b_att) ----------
            for ns in range(4):
                ect = wspool.tile([128, 4, 512], bf16, name="ect")
                nc.sync.dma_start(out=ect, in_=d_encT[:, :, ns * 512:(ns + 1) * 512])
                for kc in range(4):
                    pt = ps.tile([128, 512], f32, name="ppt", tag="big", bufs=2)
                    for hc in range(4):
                        nc.tensor.matmul(
                            pt,
                            wenc[:, hc, kc * 128:(kc + 1) * 128],
                            ect[:, hc, :],
                            start=(hc == 0), stop=(hc == 3),
                        )
                    nc.scalar.activation(
                        P_sb[kc][:, ns * 512:(ns + 1) * 512], pt,
                        Act.Identity, bias=batt[:, kc:kc + 1], scale=1.0)

            # ---------- recurrence ----------
            opend = []
            for l in range(L):
                # q.T[k, b] = w_hid-blocks @ hTb : PSUM [128,(4,32)]
                qp = ps.tile([128, 4, BS], f32, name="qp", tag="qc", bufs=2)
                for kc in range(4):
                    for hc in range(4):
                        nc.tensor.matmul(
                            qp[:, kc, :],
                            whid[:, hc, kc * 128:(kc + 1) * 128],
                            hTb[:, hc, :],
                            start=(hc == 0), stop=(hc == 3))
                q_sb = wpool.tile([128, 4, BS], bf16, name="q_sb")
                nc.vector.tensor_copy(q_sb, qp)

                # S = tanh(P + q) ; bcast q over t via stride-0 AP
                for kc in range(4):
                    qb = q_sb[:, kc, None, :].broadcast_to([128, T, BS])
                    nc.vector.tensor_tensor(
                        S_sb[kc].rearrange("p (t b) -> p t b", b=BS),
                        P_sb[kc].rearrange("p (t b) -> p t b", b=BS),
                        qb, Alu.add)
                    nc.scalar.activation(
                        S_sb[kc], S_sb[kc], Act.Tanh,
                        bias=0.0, scale=1.0)

                # scores: v-dot, 4 col-group rows of PSUM
                scp = ps.tile([128, 512], f32, name="scp", tag="scp", bufs=1)
                for ns in range(4):
                    for kc in range(4):
                        nc.tensor.matmul(
                            scp[32 * ns:32 * ns + 32, :],
                            v4[:, kc, :],
                            S_sb[kc][:, ns * 512:(ns + 1) * 512],
                            start=(kc == 0), stop=(kc == 3),
                            tile_position=(0, 32 * ns))
                sc128 = wpool.tile([128, 512], f32, name="sc128")
                nc.vector.tensor_copy(sc128, scp)
                scb = d_scb[l % 2]
                nc.sync.dma_start(out=scb[:], in_=sc128)
                dview = scb[:].rearrange("(n r) (tt b) -> n r tt b",
                                         r=32, b=BS)[:, 0]
                sctb = wpool.tile([T, BS], f32, name="sctb")
                nc.sync.dma_start(out=sctb, in_=dview)
                trp = ps.tile([32, T], f32, name="trp", tag="qc", bufs=2)
                nc.tensor.transpose(trp, sctb, id64)
                scbt = wpool.tile([32, T], f32, name="scbt")
                nc.vector.tensor_copy(scbt, trp)

                # softmax over t (no max-sub: |score| <= ||v||_1 ~ 8, exp safe)
                wbt = wpool.tile([32, T], f32, name="wbt")
                zsum = wpool.tile([32, 1], f32, name="zsum")
                nc.scalar.activation(wbt, scbt, Act.Exp, bias=0.0, scale=1.0,
                                     accum_out=zsum)
                rz = wpool.tile([32, 1], f32, name="rz")
                nc.vector.reciprocal(rz, zsum)
                albt = wpool.tile([32, T], bf16, name="albt")
                nc.vector.tensor_scalar(albt, wbt, rz, None, Alu.mult)

                # alphaT [64, 32] via two 32x32 DVE transposes
                alT = wpool.tile([64, 32], bf16, name="alT")
                nc.vector.transpose(alT[0:32, :], albt[:, 0:32])
                nc.vector.transpose(alT[32:64, :], albt[:, 32:64])

                # scatter alphaT into block-diag Ablk (zeros persist)
                nc.vector.tensor_copy(Ablk[0:64, 0:512:34], alT[:, 0::2])
                nc.vector.tensor_copy(Ablk[64:128, 1:512:34], alT[:, 1::2])

                # ctx.T [h-chunk, b] = encS-chunks.T @ Ablk-cols
                cxp = ps.tile([128, 4, BS], f32, name="cxp", tag="qc", bufs=2)
                for mc in range(4):
                    for c in range(16):
                        nc.tensor.matmul(
                            cxp[:, mc, :],
                            encS[:, c, mc * 128:(mc + 1) * 128],
                            Ablk[:, c * 32:(c + 1) * 32],
                            start=(c == 0), stop=(c == 15))

                # xT = [ctx.T (bf16) ; embT_l]
                xT = wpool.tile([128, 6, BS], bf16, name="xT")
                nc.vector.tensor_copy(xT[:, 0:4, :], cxp)
                nc.vector.tensor_copy(xT[:, 4:6, :], embT[:, :, l, :])

                # GRU gates: grz = (Wih@x + Whh@h) for r,z ; gin/ghn separate
                grzp = ps.tile([128, 8, BS], f32, name="grzp", tag="grz", bufs=1)
                ginp = ps.tile([128, 4, BS], f32, name="ginp", tag="gin", bufs=1)
                ghnp = ps.tile([128, 4, BS], f32, name="ghnp", tag="ghn", bufs=1)
                for jc in range(8):
                    for fc in range(6):
                        nc.tensor.matmul(
                            grzp[:, jc, :],
                            WihT[:, fc, jc * 128:(jc + 1) * 128],
                            xT[:, fc, :], start=(fc == 0), stop=False)
                    for hc in range(4):
                        nc.tensor.matmul(
                            grzp[:, jc, :],
                            WhhT[:, hc, jc * 128:(jc + 1) * 128],
                            hTb[:, hc, :], start=False, stop=(hc == 3))
                for jc in range(4):
                    for fc in range(6):
                        nc.tensor.matmul(
                            ginp[:, jc, :],
                            WihT[:, fc, (8 + jc) * 128:(9 + jc) * 128],
                            xT[:, fc, :], start=(fc == 0), stop=(fc == 5))
                    for hc in range(4):
                        nc.tensor.matmul(
                            ghnp[:, jc, :],
                            WhhT[:, hc, (8 + jc) * 128:(9 + jc) * 128],
                            hTb[:, hc, :], start=(hc == 0), stop=(hc == 3))

                # gate nonlinearities (bias adds on DVE, single wide ACTs)
                grzs = wpool.tile([128, 8, BS], f32, name="grzs")
                nc.vector.tensor_tensor(
                    grzs, grzp, brz[:, :, None].broadcast_to([128, 8, BS]),
                    Alu.add)
                rzs = wpool.tile([128, 8, BS], bf16, name="rzs")
                nc.scalar.activation(rzs, grzs, Act.Sigmoid, bias=0.0,
                                     scale=1.0)
                gins = wpool.tile([128, 4, BS], f32, name="gins")
                ghns = wpool.tile([128, 4, BS], f32, name="ghns")
                nc.vector.tensor_tensor(
                    gins, ginp, bin_[:, :, None].broadcast_to([128, 4, BS]),
                    Alu.add)
                nc.vector.tensor_tensor(
                    ghns, ghnp, bhn[:, :, None].broadcast_to([128, 4, BS]),
                    Alu.add)
                npre = wpool.tile([128, 4, BS], f32, name="npre")
                nc.vector.tensor_tensor(npre, rzs[:, 0:4, :], ghns, Alu.mult)
                nc.vector.tensor_tensor(npre, npre, gins, Alu.add)
                nsb = wpool.tile([128, 4, BS], f32, name="nsb")
                nc.scalar.activation(nsb, npre, Act.Tanh, bias=0.0, scale=1.0)

                # h = n + z*(h - n)
                hmn = wpool.tile([128, 4, BS], f32, name="hmn")
                nc.vector.tensor_tensor(hmn, hT, nsb, Alu.subtract)
                nc.vector.tensor_tensor(hmn, rzs[:, 4:8, :], hmn, Alu.mult)
                nc.vector.tensor_tensor(hT, nsb, hmn, Alu.add)
                nc.vector.tensor_copy(hTb, hT)
                nc.vector.tensor_copy(hall[:, :, l * BS:(l + 1) * BS], hT)

                # interleaved out-projection, spread ~4 tiles per step
                if l % 4 == 3:
                    opend.extend((l // 4, nt) for nt in range(NT))
                burst = 4 if l < L - 1 else len(opend)
                for mc, nt in [opend.pop(0) for _ in range(min(burst, len(opend)))]:
                    wt = wspool.tile([128, 4, 512], bf16, name="wt")
                    nc.sync.dma_start(
                        out=wt, in_=d_WoT[:, :, nt * 512:(nt + 1) * 512])
                    op = ps.tile([128, 512], f32, name="op",
                                 tag="big", bufs=2)
                    for kc in range(4):
                        nc.tensor.matmul(
                            op,
                            hall[:, kc, mc * 128:(mc + 1) * 128],
                            wt[:, kc, :],
                            start=(kc == 0), stop=(kc == 3))
                    ot = opool.tile([128, 512], f32, name="ot")
                    nc.vector.tensor_copy(ot, op)
                    nc.sync.dma_start(
                        out=d_out[mc * 128:(mc + 1) * 128,
                                  nt * 512:(nt + 1) * 512],
                        in_=ot)

    return nc


def _prep_inputs(inputs):
    """Host-side: shard + lay out per-core DRAM param arrays."""
    enc = np.asarray(inputs["encoded"], np.float32)
    brg = np.asarray(inputs["encoder_bridge"], np.float32)
    tgt = np.asarray(inputs["targets"])
    emb = np.asarray(inputs["embedding"], np.float32)
    w_enc = np.asarray(inputs["w_enc"], np.float32)
    w_hid = np.asarray(inputs["w_hid"], np.float32)
    b_att = np.asarray(inputs["b_att"], np.float32)
    v_att = np.asarray(inputs["v_att"], np.float32)
    W_ih = np.asarray(inputs["W_ih"], np.float32)
    W_hh = np.asarray(inputs["W_hh"], np.float32)
    b_ih = np.asarray(inputs["b_ih"], np.float32)
    b_hh = np.asarray(inputs["b_hh"], np.float32)
    W_out = np.asarray(inputs["W_out"], np.float32)
    b_out = np.asarray(inputs["b_out"], np.float32)
    W_br = np.asarray(inputs["W_bridge"], np.float32)
    b_br = np.asarray(inputs["b_bridge"], np.float32)

    bf = ml_dtypes.bfloat16
    # shared (replicated) params
    wenc = np.ascontiguousarray(w_enc.reshape(4, 128, H).transpose(1, 0, 2)).astype(bf)
    whid = np.ascontiguousarray(w_hid.reshape(4, 128, H).transpose(1, 0, 2)).astype(bf)
    batt = np.ascontiguousarray(b_att.reshape(4, 128).T)
    v4 = np.ascontiguousarray(
        np.broadcast_to(v_att.reshape(4, 128).T[:, :, None],
                        (128, 4, 32))).astype(bf)
    WihT = np.ascontiguousarray(W_ih.T.reshape(6, 128, 1536).transpose(1, 0, 2)).astype(bf)
    WhhT = np.ascontiguousarray(W_hh.T.reshape(4, 128, 1536).transpose(1, 0, 2)).astype(bf)
    brz_f = (b_ih + b_hh)[:1024]
    brz = np.ascontiguousarray(brz_f.reshape(8, 128).T)
    bin_ = np.ascontiguousarray(b_ih[1024:].reshape(4, 128).T)
    bhn = np.ascontiguousarray(b_hh[1024:].reshape(4, 128).T)
    Wp = np.zeros((CP, H), np.float32)
    Wp[:C] = W_out
    WoT = np.ascontiguousarray(Wp.T.reshape(4, 128, CP).transpose(1, 0, 2)).astype(bf)
    id64 = np.eye(64, dtype=np.float32)

    # teacher-forced inputs + bridge (host)
    xs = np.concatenate([np.zeros((1, B), tgt.dtype), tgt[:, 1:L].T], 0)  # (L,B)
    emb_seq = emb[xs]                                    # (L,B,E)
    h0 = np.tanh(brg @ W_br.T + b_br)                    # (B,H)

    in_maps = []
    for ci in range(NCORE):
        sl = slice(ci * BS, (ci + 1) * BS)
        e = enc[:, sl, :]                                # (T,32,H)
        encTc = np.ascontiguousarray(
            e.transpose(2, 0, 1).reshape(H, NBT).reshape(4, 128, NBT)
            .transpose(1, 0, 2)).astype(bf)
        encSc = np.ascontiguousarray(
            e.transpose(1, 0, 2).reshape(16, 2, T, H).transpose(1, 2, 0, 3)
            .reshape(128, 16, H)).astype(bf)
        embTc = np.ascontiguousarray(
            emb_seq[:, sl, :].transpose(2, 0, 1).reshape(2, 128, L, BS)
            .transpose(1, 0, 2, 3)).astype(bf)
        h0Tc = np.ascontiguousarray(
            h0[sl].T.reshape(4, 128, BS).transpose(1, 0, 2))
        in_maps.append({
            "encT": encTc, "encS": encSc, "embT": embTc,
            "wenc": wenc, "whid": whid, "batt": batt, "v4": v4,
            "WihT": WihT, "WhhT": WhhT, "brz": brz, "bin_": bin_,
            "bhn": bhn, "h0T": h0Tc,
            "WoT": WoT, "id64": id64,
        })
    return in_maps


def kernel(**inputs):
    if "nc" not in _CACHE:
        nc = _build()
        fixed = _split_waits_json(nc)
        nc.to_json_bytes = lambda: fixed
        _CACHE["nc"] = nc
    nc = _CACHE["nc"]
    in_maps = _prep_inputs(inputs)
    res = run_bass_kernel_spmd(nc, in_maps, list(range(NCORE)))
    outs = []
    for ci in range(NCORE):
        o = np.asarray(res.results[ci]["out"])       # (L*BS, CP)
        outs.append(o.reshape(L, BS, CP)[:, :, :C].transpose(1, 0, 2))
    full = np.concatenate(outs, 0).astype(np.float32)  # (B, L, C)
    return full + np.asarray(inputs["b_out"], np.float32)[None, None, :]
